# revision 12
# baseline (speedup 1.0000x reference)
"""Multi-head attention kernel for 8 TRN2 NeuronCores.

Problem: bs=32, ne=20 (n=400 tokens), h=12 heads, dk=64.
  Rh = R.reshape(bs,400,12,64) per-head; Q=Rh@Wq^T, K=Rh@Wk^T, V=Rh@Wv^T
  S = Q@K^T; S -= (1-mq*mk)*1e5; alpha = softmax(S/8); O = alpha@V; O *= mq.

Strategy:
  - Batch-shard: 4 batches per core, no collectives.
  - Host pre/post: transpose R to d-major per head, fold Wq^T@Wk into one
    64x64 matrix MQK so S = Rh@MQK@Rh^T (skips Q/K projections entirely),
    precompute mask bias row (mas-1)*12500; apply row mask + V bias on host.
  - Device per (b,h), all matmuls bf16 (verified 5e-3 rel err vs 2e-2 gate):
      G  [64,400]  = MQK.T-contract of Rh^T
      V  [100,64]x4 tok-major (+ ones col)
      St [100,400]x4 k-major, col-mask folded as K=65 augmented row
      Et = exp(St/8): two strided ACT ops (2+2 banks) -> bf16
      Ot [65,400] d-major = [V|1].T @ Et slices; row 64 = softmax denom
      raw Ot + denom row DMA'd out; host does denom divide + row mask.
"""

import numpy as np

H, DK, BS, NE = 12, 64, 32, 20
N = NE * NE            # 400 tokens
NCORES = 8
BPC = BS // NCORES     # 4 batches per core
TILE = 100             # token tile (400 = 4*100)
NT = N // TILE         # 4

_CACHE = {}


def _build_graph():
    import concourse.bass as bass
    import concourse.tile as tile
    from concourse import bacc, mybir

    f32 = mybir.dt.float32
    f32r = mybir.dt.float32r
    bf16 = mybir.dt.bfloat16

    nc = bacc.Bacc("TRN2", target_bir_lowering=False, debug=False,
                   enable_asserts=False)

    Rt = nc.dram_tensor("Rt", [BPC, H, DK, N], f32, kind="ExternalInput").ap()
    Bm = nc.dram_tensor("Bm", [BPC, N], f32, kind="ExternalInput").ap()
    MQK = nc.dram_tensor("MQK", [DK, DK], f32, kind="ExternalInput").ap()
    WVt = nc.dram_tensor("WVt", [DK + 1, DK], f32, kind="ExternalInput").ap()
    Ones = nc.dram_tensor("Ones", [N], f32, kind="ExternalInput").ap()
    Out = nc.dram_tensor("Out", [BPC, H, DK + 1, N], f32,
                         kind="ExternalOutput").ap()

    NRING = 4

    with tile.TileContext(nc) as tc:
        with (
            tc.tile_pool(name="consts", bufs=1) as cpool,
            tc.tile_pool(name="rht", bufs=6) as rpool,
            tc.tile_pool(name="gt", bufs=NRING) as gpool,
            tc.tile_pool(name="et", bufs=4) as epool,
            tc.tile_pool(name="vk", bufs=NRING) as vpool,
            tc.tile_pool(name="osb", bufs=4) as opool,
            tc.tile_pool(name="ps_g", bufs=1, space="PSUM") as ps_g,

            tc.tile_pool(name="ps_s", bufs=2, space="PSUM") as ps_s,
            tc.tile_pool(name="ps_o", bufs=3, space="PSUM") as ps_o,
        ):
            mqk_raw = cpool.tile([DK, DK], f32, tag="mqk_raw")
            nc.sync.dma_start(mqk_raw[:], MQK[:])
            mqk_b = cpool.tile([DK, DK], bf16, tag="mqk_b")
            nc.gpsimd.tensor_copy(mqk_b[:], mqk_raw[:])
            wvt_raw = cpool.tile([DK + 1, DK], f32, tag="wvt_raw")
            nc.sync.dma_start(wvt_raw[:], WVt[:])
            wvt_b = cpool.tile([DK + 1, DK], bf16, tag="wvt_b")
            nc.gpsimd.tensor_copy(wvt_b[:], wvt_raw[:])
            ones_raw = cpool.tile([1, N], f32, tag="ones_raw")
            nc.sync.dma_start(ones_raw[:], Ones.rearrange("(o n) -> o n", o=1))
            ones_b = cpool.tile([1, N], bf16, tag="ones_b")
            nc.gpsimd.tensor_copy(ones_b[:], ones_raw[:])
            onesb_raw = cpool.tile([TILE, NT], f32, tag="onesb_raw")
            nc.sync.dma_start(onesb_raw[:], Ones.rearrange("(s p) -> p s", p=TILE))
            onesb = cpool.tile([TILE, NT], bf16, tag="onesb")
            nc.gpsimd.tensor_copy(onesb[:], onesb_raw[:])

            # persistent ring tiles: ones rows/cols written once
            gts, vks = [], []
            for i in range(NRING):
                g = gpool.tile([DK + 1, N + DK], bf16, tag=f"gt{i}")
                nc.gpsimd.tensor_copy(g[DK:DK + 1, 0:N], ones_b[:])
                nc.gpsimd.tensor_copy(g[:, N:N + DK], wvt_b[:])
                gts.append(g)
                v = vpool.tile([TILE, NT * (DK + 1)], bf16, tag=f"vk{i}")
                nc.gpsimd.tensor_copy(
                    v[:].rearrange("p (t c) -> p t c", c=DK + 1)[:, :, DK:DK + 1],
                    onesb[:].rearrange("p (s o) -> p s o", o=1))
                vks.append(v)

            it = 0
            for b in range(BPC):
                for h in range(H):
                    gt, vk = gts[it % NRING], vks[it % NRING]
                    it += 1
                    # ---- load Rh^T (d-major) + mask-bias row (K=65 augment)
                    rht_raw = rpool.tile([DK + 1, N], f32, tag="rht_raw")
                    nc.sync.dma_start(rht_raw[0:DK, :], Rt[b, h])
                    nc.sync.dma_start(rht_raw[DK:DK + 1, :], Bm[b:b + 1, :])
                    rht_b = rpool.tile([DK + 1, N], bf16, tag="rht_b")
                    nc.gpsimd.tensor_copy(rht_b[:], rht_raw[:])

                    # ---- Gt[j,q] = sum_i MQK[i,j]*Rht[i,q]
                    g_ps = ps_g.tile([DK, N], f32, tag="g")
                    nc.tensor.matmul(g_ps[:], mqk_b[:], rht_b[0:DK, :],
                                     start=True, stop=True)
                    nc.vector.tensor_copy(gt[0:DK, 0:N], g_ps[:])

                    # ---- St (k-major) + fused V columns: rhs [65, 464]
                    # cols 0:400 = gt (St), cols 400:464 = [WVt;0] -> V tile.
                    # two 2-bank halves so ACT(h) overlaps St-matmuls(h+1)
                    et = epool.tile([TILE, NT * N], bf16, tag="et")
                    for half in range(2):
                        s_ps = ps_s.tile([TILE, 2 * 512], f32, tag="s")
                        for j in range(2):
                            t = half * 2 + j
                            nc.tensor.matmul(
                                s_ps[:, j * 512:j * 512 + N + DK],
                                rht_b[:, t * TILE:(t + 1) * TILE],
                                gt[:], start=True, stop=True)
                        nc.scalar.activation(
                            et[:, half * 2 * N:(half + 1) * 2 * N].rearrange(
                                "p (t c) -> p t c", c=N),
                            s_ps[:].rearrange(
                                "p (t c) -> p t c", c=512)[:, :, 0:N],
                            bass.mybir.ActivationFunctionType.Exp,
                            scale=0.125)
                        nc.vector.tensor_copy(
                            vk[:].rearrange(
                                "p (t c) -> p t c",
                                c=DK + 1)[:, 2 * half:2 * half + 2, 0:DK],
                            s_ps[:].rearrange(
                                "p (t c) -> p t c", c=512)[:, :, N:N + DK])

                    # ---- Ot [65,400] d-major; row 64 = softmax denominator
                    o_ps = ps_o.tile([DK + 1, N], f32, tag="o")
                    for t in range(NT):
                        nc.tensor.matmul(
                            o_ps[:],
                            vk[:, t * (DK + 1):(t + 1) * (DK + 1)],
                            et[:, t * N:(t + 1) * N],
                            start=(t == 0), stop=(t == NT - 1))

                    # ---- raw Ot + denom row out; host divides + masks
                    o_sb = opool.tile([DK + 1, N], f32, tag="o_sb")
                    nc.vector.tensor_copy(o_sb[:], o_ps[:])
                    nc.scalar.dma_start(Out[b, h], o_sb[:])

    nc.compile()
    return nc


def _get_graph():
    if "nc" not in _CACHE:
        _CACHE["nc"] = _build_graph()
    return _CACHE["nc"]


def _host_prep(R, R_mas, WQ_w, WK_w, WV_w):
    """Returns per-core input maps (host-side layout transforms are free)."""
    MQK = (WQ_w.astype(np.float64).T @ WK_w.astype(np.float64)).astype(np.float32)
    WVt = np.ascontiguousarray(
        np.vstack([WV_w.T.astype(np.float32),
                   np.zeros((1, DK), np.float32)]))
    in_maps = []
    for c in range(NCORES):
        Rc = R[c * BPC:(c + 1) * BPC]                       # [4,20,20,768]
        Rt = np.ascontiguousarray(
            Rc.reshape(BPC, N, H, DK).transpose(0, 2, 3, 1)  # [4,12,64,400]
        ).astype(np.float32)
        mas = R_mas[c * BPC:(c + 1) * BPC].reshape(BPC, N).astype(np.float32)
        Bm = ((mas - 1.0) * 12500.0).astype(np.float32)
        in_maps.append({"Rt": Rt, "Bm": Bm, "MQK": MQK, "WVt": WVt,
                        "Ones": np.ones(N, dtype=np.float32)})
    return in_maps


def kernel(R, R_mas, WQ_w, WQ_b, WK_w, WK_b, WV_w, WV_b, **kwargs):
    from concourse.bass_utils import run_bass_kernel_spmd

    R = np.asarray(R)
    R_mas = np.asarray(R_mas)
    nc = _get_graph()
    in_maps = _host_prep(R, R_mas, np.asarray(WQ_w), np.asarray(WK_w),
                         np.asarray(WV_w))
    res = run_bass_kernel_spmd(nc, in_maps, core_ids=list(range(NCORES)))
    outs = [res.results[i]["Out"] for i in range(NCORES)]     # [4,12,65,400]
    arr = np.concatenate(outs, axis=0)                        # [32,12,65,400]
    o_raw = arr[:, :, :DK, :]                                 # [32,12,64,400]
    denom = arr[:, :, DK, :]                                  # [32,12,400]
    mas = R_mas.reshape(BS, 1, N).astype(np.float32)
    scale = mas / np.maximum(denom, 1e-30)                    # [32,12,400]
    full = o_raw * scale[:, :, None, :]                       # [32,12,64,400]
    full = full.transpose(0, 3, 1, 2)                         # [32,400,12,64]
    bv = np.asarray(WV_b, dtype=np.float32)
    if np.any(bv):
        full = (full + bv[None, None, None, :]) * R_mas.reshape(BS, N, 1, 1)
    return np.ascontiguousarray(full.reshape(BS, NE, NE, H * DK),
                                dtype=np.float32)
